# revision 1
# baseline (speedup 1.0000x reference)
"""Trainium2 Bass kernel for Llama-style GQA attention (T=2048, HID=4096,
H=32 q-heads, KV=8 kv-heads, D=128), tensor-parallel over heads on 8 cores.

Per-core work (core c):
  - QKV projection for its 4 q-heads + 1 kv-head (K and V) with RoPE fused
    into the PSUM drains.
  - Causal attention for its 4 heads, computed as scores^T [s, q] so that
    softmax-normalized P tiles feed the PV matmul directly (no transposes)
    and the PV output [d, q] is exactly the lhsT layout o_proj needs.
    Softmax skips the max-subtraction (scores are O(10), exp is safe in
    fp32) and gets denominators from a ones-stationary matmul.
  - Partial o_proj: attn^T(local heads) x Wo^T(local rows) -> [T, HID]
    partial sum.  Host adds the 8 partials (the "all-reduce").

All matmuls use the float32r dtype view (full PE rate for moving dim >=
256; plain fp32 runs at 1/4 rate).
"""

import numpy as np

import concourse.bass as bass
import concourse.bacc as bacc
import concourse.mybir as mybir
import concourse.tile as tile
from concourse import bass_utils
from concourse.masks import make_identity

T = 2048
HID = 4096
H = 32
KVH = 8
D = 128
NCORES = 8
HPC = H // NCORES          # q-heads per core = 4
THETA = 10000.0
F32 = mybir.dt.float32
F32R = mybir.dt.float32r
SCALE = float(D) ** -0.5

# QKV projection output blocks per core: 4 q-heads, 1 k-head, 1 v-head
NB = HPC + 2               # 6 blocks of 128
NQK = HPC + 1              # blocks 0..4 get RoPE (Q0..Q3, K); block 5 is V

TQ = 256                   # QKV t-chunk width (8 chunks)
NTQ = T // TQ
KCH = HID // 128           # 32 contraction chunks


def _pieces(lo, hi, step=512):
    """Split [lo, hi) at multiples of `step` (PSUM-bank-aligned chunks)."""
    out = []
    while lo < hi:
        nxt = min(hi, (lo // step + 1) * step)
        out.append((lo, nxt))
        lo = nxt
    return out


def build_nc(loop_n=1, phases=3):
    nc = bacc.Bacc("TRN2", target_bir_lowering=False, debug=False,
                   num_devices=NCORES)

    hT = nc.dram_tensor("hT", [HID, T], F32R, kind="ExternalInput").ap()
    wqkvT = nc.dram_tensor("wqkvT", [HID, NB * D], F32R, kind="ExternalInput").ap()
    woT = nc.dram_tensor("woT", [HPC * D, HID], F32R, kind="ExternalInput").ap()
    # cos2 = [cos; cos], sinm2 = [-sin; +sin] stacked along d (see host_inputs)
    cosT = nc.dram_tensor("cosT", [D, T], F32, kind="ExternalInput").ap()
    sinT = nc.dram_tensor("sinT", [D, T], F32, kind="ExternalInput").ap()
    trim = nc.dram_tensor("trim", [128, 128], F32R, kind="ExternalInput").ap()
    onesm = nc.dram_tensor("onesm", [128, 128], F32R, kind="ExternalInput").ap()
    out = nc.dram_tensor("out", [T, HID], F32, kind="ExternalOutput").ap()

    # DRAM scratch: roped q/k heads [5*128, T] and transposed V [T, 128]
    qk_dram = nc.dram_tensor("qk_dram", [NQK * D, T], F32R, kind="Internal").ap()
    v_dram = nc.dram_tensor("v_dram", [T, D], F32R, kind="Internal").ap()

    import contextlib

    with tile.TileContext(nc) as tc, contextlib.ExitStack() as _loopctx:
        if loop_n > 1:
            _loopctx.enter_context(tc.For_i(0, loop_n))
        # ---------------- Phase 1: QKV projection + RoPE + V transpose ----
        with tc.tile_pool(name="qkvconst", bufs=1) as cpool, \
             tc.tile_pool(name="wq", bufs=1) as wpool, \
             tc.tile_pool(name="hid", bufs=2) as hpool, \
             tc.tile_pool(name="qkvstage", bufs=3) as spool, \
             tc.tile_pool(name="qkvpsum", bufs=2, space="PSUM") as qpsum, \
             tc.tile_pool(name="trpsum", bufs=2, space="PSUM") as tpsum:

            ident = cpool.tile([128, 128], F32)
            make_identity(nc, ident)
            cos_sb = cpool.tile([D, T], F32)
            sin_sb = cpool.tile([D, T], F32)
            for tq in range(NTQ):
                sl = slice(tq * TQ, (tq + 1) * TQ)
                nc.sync.dma_start(cos_sb[:, sl], cosT[:, sl])
                nc.sync.dma_start(sin_sb[:, sl], sinT[:, sl])

            wq = []
            for k in range(KCH):
                wt = wpool.tile([128, NB * D], F32R, name=f"wq{k}", tag=f"wq{k}")
                for nb in range(NB):
                    sl = slice(nb * D, (nb + 1) * D)
                    nc.sync.dma_start(wt[:, sl],
                                      wqkvT[k * 128:(k + 1) * 128, sl])
                wq.append(wt)

            for tq in range(NTQ):
                tlo = tq * TQ
                hk = []
                for k in range(KCH):
                    ht = hpool.tile([128, TQ], F32R, name=f"hk{k}", tag=f"hk{k}")
                    nc.sync.dma_start(ht, hT[k * 128:(k + 1) * 128, tlo:tlo + TQ])
                    hk.append(ht)
                for nb in range(NB):
                    ps = qpsum.tile([128, TQ], F32, name="qkv_ps", tag="qkv_ps")
                    for k in range(KCH):
                        nc.tensor.matmul(
                            ps, wq[k][:, nb * D:(nb + 1) * D], hk[k],
                            start=(k == 0), stop=(k == KCH - 1))
                    if nb < NQK:
                        # RoPE: out = raw*cos2 + swap_halves(raw)*[-sin;+sin]
                        # (DVE lanes are partition-fixed, so the half swap
                        # goes through an SBUF->SBUF DMA.)
                        raw = spool.tile([128, TQ], F32, name="roperaw",
                                         tag="roperaw")
                        nc.scalar.copy(raw, ps)
                        sw = spool.tile([128, TQ], F32, name="ropesw",
                                        tag="ropesw")
                        nc.sync.dma_start(sw[0:64], raw[64:128])
                        nc.sync.dma_start(sw[64:128], raw[0:64])
                        st = spool.tile([128, TQ], F32R, name="ropest",
                                        tag="ropest")
                        c_sl = cos_sb[:, tlo:tlo + TQ]
                        s_sl = sin_sb[:, tlo:tlo + TQ]
                        nc.vector.tensor_mul(st, raw, c_sl)
                        nc.vector.tensor_mul(sw, sw, s_sl)
                        nc.vector.tensor_add(st, st, sw)
                        nc.sync.dma_start(
                            qk_dram[nb * D:(nb + 1) * D, tlo:tlo + TQ], st)
                    else:
                        # V: drain, transpose 128x128 tiles, store [t, d]
                        vs = spool.tile([128, TQ], F32, name="vstage", tag="vstage")
                        nc.scalar.copy(vs, ps)
                        for i in range(TQ // 128):
                            tp = tpsum.tile([128, 128], F32, name="vt_ps",
                                            tag="vt_ps")
                            nc.tensor.transpose(tp, vs[:, i * 128:(i + 1) * 128],
                                                ident)
                            vt = spool.tile([128, 128], F32R, name="vt_sb",
                                            tag="vt_sb")
                            nc.vector.tensor_copy(vt, tp)
                            nc.sync.dma_start(
                                v_dram[tlo + i * 128:tlo + (i + 1) * 128, :], vt)

        # ---------------- Phase 2+3 SBUF residents ------------------------
        if phases >= 2:
            with tc.tile_pool(name="aconst", bufs=1) as apool, \
               tc.tile_pool(name="attn_out", bufs=1) as opool, \
               tc.tile_pool(name="wo", bufs=1) as wopool:

              tri = apool.tile([128, 128], F32R)
              nc.sync.dma_start(tri, trim)
              ones = apool.tile([128, 128], F32R)
              nc.sync.dma_start(ones, onesm)
              kt = apool.tile([D, T], F32R)
              for j in range(T // 128):
                  sl = slice(j * 128, (j + 1) * 128)
                  nc.sync.dma_start(kt[:, sl], qk_dram[HPC * D:(HPC + 1) * D, sl])
              vv = apool.tile([128, T], F32R)   # [:, j*128:+128] = V[j-block] [s,d]
              for j in range(T // 128):
                  nc.sync.dma_start(vv[:, j * 128:(j + 1) * 128],
                                    v_dram[j * 128:(j + 1) * 128, :])

              wo = []
              for h in range(HPC):
                  wt = wopool.tile([128, HID], F32R, name=f"wo{h}", tag=f"wo{h}")
                  for cc in range(HID // 512):
                      sl = slice(cc * 512, (cc + 1) * 512)
                      nc.sync.dma_start(wt[:, sl], woT[h * D:(h + 1) * D, sl])
                  wo.append(wt)

              attn = []
              for h in range(HPC):
                  at = opool.tile([D, T], F32R, name=f"attn{h}", tag=f"attn{h}")
                  attn.append(at)

              # ---------------- Phase 2: attention per head -----------------
              with tc.tile_pool(name="qt", bufs=2) as qtpool, \
                   tc.tile_pool(name="pj", bufs=1) as ppool, \
                   tc.tile_pool(name="rec", bufs=2) as rpool, \
                   tc.tile_pool(name="scps", bufs=2, space="PSUM") as scps, \
                   tc.tile_pool(name="pvps", bufs=2, space="PSUM") as pvps, \
                   tc.tile_pool(name="dnps", bufs=1, space="PSUM") as dnps:

                  for h in range(HPC):
                      qt = qtpool.tile([D, T], F32R, name="qt", tag="qt")
                      for qq in range(T // 512):
                          sl = slice(qq * 512, (qq + 1) * 512)
                          nc.sync.dma_start(qt[:, sl],
                                            qk_dram[h * D:(h + 1) * D, sl])

                      for half in range(2):
                          q_lo = 1024 * half
                          js = range(8 * (half + 1))
                          pv = pvps.tile([128, 1024], F32, name="pv", tag="pv")
                          dn = dnps.tile([128, 1024], F32, name="dn", tag="dn")

                          pjs = {}
                          # scores + exp (+ causal mask on the diagonal block)
                          for j in js:
                              ls = max(0, 128 * j - q_lo)
                              pj = ppool.tile([128, 1024 - ls], F32R,
                                              name=f"p{j}", tag=f"p{j}")
                              pjs[j] = (pj, ls)
                              for (plo, phi) in _pieces(ls, 1024):
                                  w = phi - plo
                                  sc = scps.tile([128, 512], F32, name="sc",
                                                 tag="sc")
                                  nc.tensor.matmul(
                                      sc[:, :w],
                                      kt[:, j * 128:(j + 1) * 128],
                                      qt[:, q_lo + plo:q_lo + phi],
                                      start=True, stop=True)
                                  nc.scalar.activation(
                                      pj[:, plo - ls:phi - ls], sc[:, :w],
                                      mybir.ActivationFunctionType.Exp,
                                      scale=SCALE)
                              if 128 * j >= q_lo:
                                  nc.vector.tensor_mul(pj[:, 0:128],
                                                       pj[:, 0:128], tri)
                          # PV + denominator accumulation over j
                          for j in js:
                              pj, ls = pjs[j]
                              for (plo, phi) in _pieces(ls, 1024):
                                  last = (q_lo + phi) // 128 - 1
                                  kw = dict(start=(j == 0), stop=(j == last))
                                  nc.tensor.matmul(
                                      pv[:, plo:phi],
                                      vv[:, j * 128:(j + 1) * 128],
                                      pj[:, plo - ls:phi - ls], **kw)
                                  nc.tensor.matmul(
                                      dn[:, plo:phi], ones,
                                      pj[:, plo - ls:phi - ls], **kw)
                          rec = rpool.tile([128, 1024], F32, name="rec", tag="rec")
                          nc.vector.reciprocal(rec, dn)
                          nc.vector.tensor_mul(attn[h][:, q_lo:q_lo + 1024],
                                               pv, rec)

            # ---------------- Phase 3: o_proj partial ---------------------
            if phases >= 3:
                with tc.tile_pool(name="ostage", bufs=2) as ospool, \
                   tc.tile_pool(name="ops", bufs=2, space="PSUM") as opsum:
                  for tb in range(T // 128):
                      for ch in range(2):
                          ps = opsum.tile([128, 2048], F32, name="o_ps",
                                          tag="o_ps")
                          for h in range(HPC):
                              lhs = attn[h][:, tb * 128:(tb + 1) * 128]
                              for cc in range(4):
                                  cl = ch * 2048 + cc * 512
                                  nc.tensor.matmul(
                                      ps[:, cc * 512:(cc + 1) * 512],
                                      lhs, wo[h][:, cl:cl + 512],
                                      start=(h == 0), stop=(h == HPC - 1))
                          ob = ospool.tile([128, 2048], F32, name="ob", tag="ob")
                          nc.scalar.copy(ob, ps)
                          for cc in range(4):
                              nc.sync.dma_start(
                                  out[tb * 128:(tb + 1) * 128,
                                      ch * 2048 + cc * 512:
                                      ch * 2048 + (cc + 1) * 512],
                                  ob[:, cc * 512:(cc + 1) * 512])
    nc.compile()
    return nc


def host_inputs(hidden_states, positions, Wqkv, Wo):
    """Build the 8 per-core input maps (host-side sharding + layout prep)."""
    f = np.float32
    hT = np.ascontiguousarray(hidden_states.T.astype(f))
    half = D // 2
    inv_freq = 1.0 / (THETA ** (np.arange(half, dtype=np.float64) / half))
    ang = inv_freq[:, None] * positions.astype(np.float64)[None, :]
    cos = np.cos(ang).astype(f)
    sin = np.sin(ang).astype(f)
    cosT = np.vstack([cos, cos])                  # [D, T]
    sinT = np.vstack([-sin, sin])                 # rotate-half sign baked in
    trim = (np.arange(128)[:, None] <= np.arange(128)[None, :]).astype(f)

    in_maps = []
    for c in range(NCORES):
        rows = list(range(c * HPC * D, (c + 1) * HPC * D))          # Q heads
        rows += list(range(H * D + c * D, H * D + (c + 1) * D))     # K head
        rows += list(range((H + KVH) * D + c * D,
                           (H + KVH) * D + (c + 1) * D))            # V head
        wqkvT = np.ascontiguousarray(Wqkv[rows, :].T.astype(f))
        woT = np.ascontiguousarray(Wo[:, c * HPC * D:(c + 1) * HPC * D].T
                                   .astype(f))
        in_maps.append({"hT": hT, "wqkvT": wqkvT, "woT": woT,
                        "cosT": cosT, "sinT": sinT, "trim": trim,
                        "onesm": np.ones((128, 128), f)})
    return in_maps


_NC_CACHE = {}


def get_nc(loop_n=1, phases=3):
    key = (loop_n, phases)
    if key not in _NC_CACHE:
        _NC_CACHE[key] = build_nc(loop_n, phases)
    return _NC_CACHE[key]


def kernel(hidden_states, positions, Wqkv, Wo, _trace=False):
    hidden_states = np.asarray(hidden_states)
    positions = np.asarray(positions)
    Wqkv = np.asarray(Wqkv)
    Wo = np.asarray(Wo)
    in_maps = host_inputs(hidden_states, positions, Wqkv, Wo)
    nc = get_nc()
    res = bass_utils.run_bass_kernel_spmd(
        nc, in_maps, core_ids=list(range(NCORES)), trace=_trace)
    acc = np.zeros((T, HID), np.float64)
    for r in res.results:
        acc += r["out"].astype(np.float64)
    out = acc.astype(np.float32)
    if _trace:
        return out, res
    return out



# revision 23
# speedup vs baseline: 253.8660x; 253.8660x over previous
"""Trainium2 Bass kernel for Llama-style GQA attention (T=2048, HID=4096,
H=32 q-heads, KV=8 kv-heads, D=128), tensor-parallel over heads on 8 cores.

Per-core work (core c):
  - QKV projection for its 4 q-heads + 1 kv-head (K and V), k-major over
    the contraction so the streamed h tiles free early; RoPE fused into
    the PSUM drains.  Roped q/k (fp32r) and PE-transposed V (bf16) stay
    SBUF-resident -- no DRAM scratch roundtrip.  (An xbar dma_transpose
    for V passed CoreSim but produced wrong data on hardware; PE
    transpose is the validated path.)
  - Causal attention per head as scores^T [s, q]: per s-block j the
    score matmul, exp (Activation), causal mask + denominator
    accumulation (DVE, fp16), and PV matmul (PE) pipeline so the
    Activation-bound exp overlaps the PE work.  Softmax skips the
    max-subtraction (scores are O(10), exp is safe in fp32); the
    denominator is a ones-stationary matmul over the DVE-accumulated
    column sums.  P/V are bf16 (same PE rate, half the SBUF cost).
  - Partial o_proj: attn^T(local heads) x Wo^T(local rows) -> [T, HID]
    partial sum.  Host adds the 8 partials (the "all-reduce").

DMA discipline: every dma_start costs ~0.6us on the serial HWDGE +
sequencer, so transfers are batched into few multi-dim descriptors
(~80 total).  Weight/h loads are interleaved in issue order so the first
QKV matmuls are not stuck behind the full 12.6MB weight transfer.
"""

import numpy as np
import ml_dtypes

import concourse.bass as bass
import concourse.bacc as bacc
import concourse.mybir as mybir
import concourse.tile as tile
from concourse import bass_utils
from concourse.masks import make_identity

T = 2048
HID = 4096
H = 32
KVH = 8
D = 128
NCORES = 8
HPC = H // NCORES          # q-heads per core = 4
THETA = 10000.0
F32 = mybir.dt.float32
F32R = mybir.dt.float32r
BF16 = mybir.dt.bfloat16
FP16 = mybir.dt.float16
SCALE = float(D) ** -0.5

# QKV projection output blocks per core: 4 q-heads, 1 k-head, 1 v-head
NB = HPC + 2               # 6 blocks of 128
NQK = HPC + 1              # blocks 0..4 get RoPE (Q0..Q3, K); block 5 is V

TQ = 256                   # QKV t-chunk width (8 chunks)
NTQ = T // TQ
KCH = HID // 128           # 32 contraction chunks
KG = 8                     # wq DMA groups (4 k-chunks each)


def _pieces(lo, hi, step=512):
    """Split [lo, hi) at multiples of `step` (PSUM-bank-aligned chunks)."""
    out = []
    while lo < hi:
        nxt = min(hi, (lo // step + 1) * step)
        out.append((lo, nxt))
        lo = nxt
    return out


def build_nc(loop_n=1, phases=3):
    nc = bacc.Bacc("TRN2", target_bir_lowering=False, debug=False,
                   num_devices=NCORES)

    hT = nc.dram_tensor("hT", [HID, T], F32R, kind="ExternalInput").ap()
    wqkvT = nc.dram_tensor("wqkvT", [HID, NB * D], F32R,
                           kind="ExternalInput").ap()
    woT = nc.dram_tensor("woT", [HPC * D, HID], F32R, kind="ExternalInput").ap()
    # cos2 = [cos; cos], sinm2 = [-sin; +sin] stacked along d (see host_inputs)
    cosT = nc.dram_tensor("cosT", [D, T], F32, kind="ExternalInput").ap()
    sinT = nc.dram_tensor("sinT", [D, T], F32, kind="ExternalInput").ap()
    trim = nc.dram_tensor("trim", [128, 128], BF16, kind="ExternalInput").ap()
    onesm = nc.dram_tensor("onesm", [128, 128], FP16, kind="ExternalInput").ap()
    out = nc.dram_tensor("out", [T, HID], F32, kind="ExternalOutput").ap()

    import contextlib

    with tile.TileContext(nc) as tc, contextlib.ExitStack() as _loopctx:
        if loop_n > 1:
            _loopctx.enter_context(tc.For_i(0, loop_n))

        with contextlib.ExitStack() as _resctx:
            rpool = _resctx.enter_context(tc.tile_pool(name="res", bufs=1))
            # SBUF residents spanning phases: roped q heads [d, t], roped k
            # [d, t], v in block-column layout ([s_local, d] per 128-block).
            q_res = [rpool.tile([D, T], F32R, name=f"qres{h}", tag=f"qres{h}")
                     for h in range(HPC)]
            k_res = rpool.tile([D, T], F32R)
            v_res = rpool.tile([128, T], BF16)
            tri = rpool.tile([128, 128], BF16)
            nc.sync.dma_start(tri, trim)
            ones = rpool.tile([128, 128], FP16)
            nc.sync.dma_start(ones, onesm)
            ident = rpool.tile([128, 128], F32)
            make_identity(nc, ident)

            # ------------- Phase 1: QKV projection + RoPE + V transpose ----
            with tc.tile_pool(name="wq", bufs=1) as wpool, \
                 tc.tile_pool(name="hid", bufs=1) as hpool, \
                 tc.tile_pool(name="cs", bufs=2) as cspool, \
                 tc.tile_pool(name="stage", bufs=2) as stpool, \
                 tc.tile_pool(name="swap", bufs=2) as swpool, \
                 tc.tile_pool(name="vstage", bufs=2) as vspool, \
                 tc.tile_pool(name="qkvpsum", bufs=1, space="PSUM") as qpsum, \
                 tc.tile_pool(name="trpsum", bufs=2, space="PSUM") as tpsum:

                # weights: 8 group tiles [128, 4x768], one DMA each
                KPG = KCH // KG          # k-chunks per group = 4
                GW = NB * D * KPG        # columns per group tile
                wq = []
                for g in range(KG):
                    wt = wpool.tile([128, GW], F32R, name=f"wqg{g}",
                                    tag=f"wqg{g}")
                    wq.append(wt)

                def wq_dma(g):
                    dst = wq[g].rearrange("p (k n) -> p k n", k=KPG)
                    src = wqkvT[g * 512:(g + 1) * 512, :] \
                        .rearrange("(k p) n -> p k n", p=128)
                    nc.sync.dma_start(dst, src)

                def wq_sl(k, nb):
                    o = (k % KPG) * NB * D + nb * D
                    return wq[k // KPG][:, o:o + D]

                def chunk_inputs(tq):
                    tlo = tq * TQ
                    hq = []
                    for qd in range(4):
                        ht = hpool.tile([128, 8 * TQ], F32R, name=f"hq{qd}",
                                        tag=f"hq{qd}")
                        dst = ht.rearrange("p (k t) -> p k t", k=8)
                        src = hT[qd * 1024:(qd + 1) * 1024, tlo:tlo + TQ] \
                            .rearrange("(k p) t -> p k t", p=128)
                        nc.sync.dma_start(dst, src)
                        hq.append(ht)
                    cos_t = cspool.tile([128, TQ], F32, name="cos", tag="cos")
                    nc.sync.dma_start(cos_t, cosT[:, tlo:tlo + TQ])
                    sin_t = cspool.tile([128, TQ], F32, name="sin", tag="sin")
                    nc.sync.dma_start(sin_t, sinT[:, tlo:tlo + TQ])
                    return hq, cos_t, sin_t

                # issue order: first weight group, chunk-0 inputs, the rest
                # of the weights (so chunk-0 compute streams with the load)
                wq_dma(0)
                nxt = chunk_inputs(0)
                for g in range(1, KG):
                    wq_dma(g)

                for tq in range(NTQ):
                    tlo = tq * TQ
                    hq, cos_t, sin_t = nxt
                    if tq + 1 < NTQ:
                        nxt = chunk_inputs(tq + 1)

                    # one PSUM bank per output block (accumulation groups
                    # must be bank-exclusive)
                    pss = [qpsum.tile([128, TQ], F32, name=f"qkv_ps{nb}",
                                      tag=f"qkv_ps{nb}") for nb in range(NB)]

                    for k in range(KCH):
                        hsl = hq[k // 8][:, (k % 8) * TQ:(k % 8 + 1) * TQ]
                        for nb in range(NB):
                            nc.tensor.matmul(
                                pss[nb], wq_sl(k, nb), hsl,
                                start=(k == 0), stop=(k == KCH - 1))

                    raw5 = stpool.tile([128, NQK * TQ], F32, name="raw5",
                                       tag="raw5")
                    sw5 = swpool.tile([128, NQK * TQ], F32, name="sw5",
                                      tag="sw5")
                    for nb in range(NQK):
                        nc.scalar.copy(raw5[:, nb * TQ:(nb + 1) * TQ],
                                       pss[nb])
                    # V: drain, transpose 128x128 tiles on PE, store [s, d]
                    vs = vspool.tile([128, TQ], F32, name="vs", tag="vs")
                    nc.scalar.copy(vs, pss[NQK])
                    for i in range(TQ // 128):
                        tp = tpsum.tile([128, 128], F32, name="vt_ps",
                                        tag="vt_ps")
                        nc.tensor.transpose(tp, vs[:, i * 128:(i + 1) * 128],
                                            ident)
                        j = 2 * tq + i
                        nc.vector.tensor_copy(v_res[:, j * 128:(j + 1) * 128],
                                              tp)

                    # RoPE half-swap for all 5 blocks in 2 batched DMAs
                    # (DVE lanes are partition-fixed, so the d-half swap goes
                    # through SBUF->SBUF DMA).
                    r3 = raw5.rearrange("p (b t) -> p b t", b=NQK)
                    s3 = sw5.rearrange("p (b t) -> p b t", b=NQK)
                    nc.sync.dma_start(s3[0:64], r3[64:128])
                    nc.sync.dma_start(s3[64:128], r3[0:64])
                    for b in range(NQK):
                        bs = slice(b * TQ, (b + 1) * TQ)
                        nc.vector.tensor_mul(sw5[:, bs], sw5[:, bs], sin_t)
                        nc.vector.tensor_mul(raw5[:, bs], raw5[:, bs], cos_t)
                        dst = q_res[b] if b < HPC else k_res
                        nc.vector.tensor_add(dst[:, tlo:tlo + TQ],
                                             raw5[:, bs], sw5[:, bs])

            # ---------------- Phase 2: attention per head ------------------
            if phases >= 2:
                opool = _resctx.enter_context(
                    tc.tile_pool(name="attn_out", bufs=1))
                wopool = _resctx.enter_context(
                    tc.tile_pool(name="wo", bufs=1))
                wo = []
                for h in range(HPC):
                    wt = wopool.tile([128, HID], F32R, name=f"wo{h}",
                                     tag=f"wo{h}")
                    nc.sync.dma_start(wt, woT[h * D:(h + 1) * D, :])
                    wo.append(wt)
                attn = []
                for h in range(HPC):
                    at = opool.tile([D, T], F32R, name=f"attn{h}",
                                    tag=f"attn{h}")
                    attn.append(at)

                with tc.tile_pool(name="pj", bufs=1) as ppool, \
                     tc.tile_pool(name="dna", bufs=2) as dpool, \
                     tc.tile_pool(name="rec", bufs=2) as rpool2, \
                     tc.tile_pool(name="scps", bufs=2, space="PSUM") as scps, \
                     tc.tile_pool(name="pvps", bufs=2, space="PSUM") as pvps:

                    # The two causal halves of a head are independent
                    # dependency chains; interleaving them (2 units of half1
                    # per unit of half0) keeps PE/Act/DVE all fed despite the
                    # serial sc -> exp -> mask -> pv chain within each unit.
                    for h in range(HPC):
                        qt = q_res[h]

                        class _Stream:
                            pass

                        def make_stream(s):
                            st = _Stream()
                            st.s = s
                            st.q_lo = 1024 * s
                            st.pv = pvps.tile([128, 1024], F32,
                                              name=f"pv{s}", tag="pv")
                            st.dna = dpool.tile([128, 1024], FP16,
                                                name=f"dna{s}", tag=f"dna{s}")
                            st.prev = None
                            st.pj_all = []
                            return st

                        def emit_pv(st, j, le, pj):
                            # far-from-diagonal pieces first: they only
                            # depend on exp, not on the tri-mask mul
                            for (plo, phi) in reversed(_pieces(le, 1024)):
                                last = (st.q_lo + phi) // 128 - 1
                                nc.tensor.matmul(
                                    st.pv[:, plo:phi],
                                    v_res[:, j * 128:(j + 1) * 128],
                                    pj[:, plo - le:phi - le],
                                    start=(j == 0), stop=(j == last))

                        def emit_unit(st, j):
                            ls = max(0, 128 * j - st.q_lo)
                            # extend so every matmul piece is >=256 wide
                            # (narrow fp32r matmuls run at 1/4 rate); the
                            # extension columns are zeroed after exp.
                            le = ls - 128 if ls % 512 == 384 else ls
                            pj = ppool.tile([128, 1024 - le], BF16,
                                            name=f"p{st.s}_{j}",
                                            tag=f"p{st.s}_{j}")
                            sc = scps.tile([128, 1024], F32,
                                           name=f"sc{st.s}", tag="sc")
                            for (plo, phi) in _pieces(le, 1024):
                                nc.tensor.matmul(
                                    sc[:, plo:phi],
                                    k_res[:, j * 128:(j + 1) * 128],
                                    qt[:, st.q_lo + plo:st.q_lo + phi],
                                    start=True, stop=True)
                            nc.scalar.activation(
                                pj, sc[:, le:1024],
                                mybir.ActivationFunctionType.Exp,
                                scale=SCALE)
                            if le < ls:
                                nc.vector.memset(pj[:, 0:ls - le], 0.0)
                            if 128 * j >= st.q_lo:
                                o = ls - le
                                nc.vector.tensor_mul(
                                    pj[:, o:o + 128], pj[:, o:o + 128], tri)
                            # software pipeline: PV of the previous j comes
                            # after this j's score matmuls, so PE always has
                            # mask-independent work queued while the
                            # Activation engine runs exp.
                            if st.prev is not None:
                                emit_pv(st, *st.prev)
                            st.prev = (j, le, pj)
                            st.pj_all.append((le, pj))

                        def finish_stream(st):
                            emit_pv(st, *st.prev)
                            # denominator column-sums batched at stream end
                            # so they stay off the per-j critical cadence
                            for i, (le, pj) in enumerate(st.pj_all):
                                if i == 0:
                                    nc.vector.tensor_copy(st.dna, pj)
                                else:
                                    nc.vector.tensor_add(
                                        st.dna[:, le:1024],
                                        st.dna[:, le:1024], pj)
                            dn = scps.tile([128, 1024], F32, name=f"dn{st.s}",
                                           tag="sc")
                            for (plo, phi) in _pieces(0, 1024):
                                nc.tensor.matmul(dn[:, plo:phi], ones,
                                                 st.dna[:, plo:phi],
                                                 start=True, stop=True)
                            rec = rpool2.tile([128, 1024], F32,
                                              name=f"rec{st.s}",
                                              tag=f"rec{st.s}")
                            nc.vector.reciprocal(rec, dn)
                            nc.vector.tensor_mul(
                                attn[h][:, st.q_lo:st.q_lo + 1024],
                                st.pv, rec)

                        for s in range(2):
                            st = make_stream(s)
                            for j in range(8 * (s + 1)):
                                emit_unit(st, j)
                            finish_stream(st)

                # ---------------- Phase 3: o_proj partial ------------------
                if phases >= 3:
                    with tc.tile_pool(name="ostage", bufs=3) as ospool, \
                         tc.tile_pool(name="ops", bufs=2,
                                      space="PSUM") as opsum:
                        for tb in range(T // 128):
                            for ch in range(2):
                                ps = opsum.tile([128, 2048], F32, name="o_ps",
                                                tag="o_ps")
                                for h in range(HPC):
                                    lhs = attn[h][:, tb * 128:(tb + 1) * 128]
                                    for cc in range(4):
                                        cl = ch * 2048 + cc * 512
                                        nc.tensor.matmul(
                                            ps[:, cc * 512:(cc + 1) * 512],
                                            lhs, wo[h][:, cl:cl + 512],
                                            start=(h == 0),
                                            stop=(h == HPC - 1))
                                ob = ospool.tile([128, 2048], F32, name="ob",
                                                 tag="ob")
                                orow = out[tb * 128:(tb + 1) * 128,
                                           ch * 2048:(ch + 1) * 2048]
                                if tb == T // 128 - 1 and ch == 1:
                                    # pipeline the drain of the very last
                                    # tile so the kernel tail is short
                                    for q4 in range(4):
                                        sl = slice(q4 * 512, (q4 + 1) * 512)
                                        nc.scalar.copy(ob[:, sl], ps[:, sl])
                                        nc.sync.dma_start(orow[:, sl],
                                                          ob[:, sl])
                                else:
                                    nc.scalar.copy(ob, ps)
                                    nc.sync.dma_start(orow, ob)
    nc.compile()
    return nc


def host_inputs(hidden_states, positions, Wqkv, Wo):
    """Build the 8 per-core input maps (host-side sharding + layout prep)."""
    f = np.float32
    hT = np.ascontiguousarray(hidden_states.T.astype(f))
    half = D // 2
    inv_freq = 1.0 / (THETA ** (np.arange(half, dtype=np.float64) / half))
    ang = inv_freq[:, None] * positions.astype(np.float64)[None, :]
    cos = np.cos(ang).astype(f)
    sin = np.sin(ang).astype(f)
    cosT = np.vstack([cos, cos])                  # [D, T]
    sinT = np.vstack([-sin, sin])                 # rotate-half sign baked in
    trim = (np.arange(128)[:, None] <= np.arange(128)[None, :]) \
        .astype(ml_dtypes.bfloat16)
    onesm = np.ones((128, 128), np.float16)

    in_maps = []
    for c in range(NCORES):
        rows = list(range(c * HPC * D, (c + 1) * HPC * D))          # Q heads
        rows += list(range(H * D + c * D, H * D + (c + 1) * D))     # K head
        rows += list(range((H + KVH) * D + c * D,
                           (H + KVH) * D + (c + 1) * D))            # V head
        wqkvT = np.ascontiguousarray(Wqkv[rows, :].T.astype(f))
        woT = np.ascontiguousarray(Wo[:, c * HPC * D:(c + 1) * HPC * D].T
                                   .astype(f))
        in_maps.append({"hT": hT, "wqkvT": wqkvT, "woT": woT,
                        "cosT": cosT, "sinT": sinT, "trim": trim,
                        "onesm": onesm})
    return in_maps


_NC_CACHE = {}


def get_nc(loop_n=1, phases=3):
    key = (loop_n, phases)
    if key not in _NC_CACHE:
        _NC_CACHE[key] = build_nc(loop_n, phases)
    return _NC_CACHE[key]


def kernel(hidden_states, positions, Wqkv, Wo, _trace=False):
    hidden_states = np.asarray(hidden_states)
    positions = np.asarray(positions)
    Wqkv = np.asarray(Wqkv)
    Wo = np.asarray(Wo)
    in_maps = host_inputs(hidden_states, positions, Wqkv, Wo)
    nc = get_nc()
    res = bass_utils.run_bass_kernel_spmd(
        nc, in_maps, core_ids=list(range(NCORES)), trace=_trace)
    acc = np.zeros((T, HID), np.float64)
    for r in res.results:
        acc += r["out"].astype(np.float64)
    out = acc.astype(np.float32)
    if _trace:
        return out, res
    return out


# revision 26
# speedup vs baseline: 267.8061x; 1.0549x over previous
"""Trainium2 Bass kernel for Llama-style GQA attention (T=2048, HID=4096,
H=32 q-heads, KV=8 kv-heads, D=128), tensor-parallel over heads on 8 cores.

Per-core work (core c):
  - QKV projection for its 4 q-heads + 1 kv-head (K and V), k-major over
    the contraction so the streamed h tiles free early; RoPE fused into
    the PSUM drains.  Roped q/k (fp32r) and PE-transposed V (bf16) stay
    SBUF-resident -- no DRAM scratch roundtrip.  (An xbar dma_transpose
    for V passed CoreSim but produced wrong data on hardware; PE
    transpose is the validated path.)
  - Causal attention per head as scores^T [s, q]: per s-block j the
    score matmul, exp (Activation), causal mask + denominator
    accumulation (DVE, fp16), and PV matmul (PE) pipeline so the
    Activation-bound exp overlaps the PE work.  Softmax skips the
    max-subtraction (scores are O(10), exp is safe in fp32); the
    denominator is a ones-stationary matmul over the DVE-accumulated
    column sums.  P/V are bf16 (same PE rate, half the SBUF cost).
  - Partial o_proj: attn^T(local heads) x Wo^T(local rows) -> [T, HID]
    partial sum.  Host adds the 8 partials (the "all-reduce").

DMA discipline: every dma_start costs ~0.6us on the serial HWDGE +
sequencer, so transfers are batched into few multi-dim descriptors
(~80 total).  Weight/h loads are interleaved in issue order so the first
QKV matmuls are not stuck behind the full 12.6MB weight transfer.
"""

import numpy as np
import ml_dtypes

import concourse.bass as bass
import concourse.bacc as bacc
import concourse.mybir as mybir
import concourse.tile as tile
from concourse import bass_utils
from concourse.masks import make_identity

T = 2048
HID = 4096
H = 32
KVH = 8
D = 128
NCORES = 8
HPC = H // NCORES          # q-heads per core = 4
THETA = 10000.0
F32 = mybir.dt.float32
F32R = mybir.dt.float32r
BF16 = mybir.dt.bfloat16
FP16 = mybir.dt.float16
SCALE = float(D) ** -0.5

# QKV projection output blocks per core: 4 q-heads, 1 k-head, 1 v-head
NB = HPC + 2               # 6 blocks of 128
NQK = HPC + 1              # blocks 0..4 get RoPE (Q0..Q3, K); block 5 is V

TQ = 256                   # QKV t-chunk width (8 chunks)
NTQ = T // TQ
KCH = HID // 128           # 32 contraction chunks
KG = 16                    # wq DMA groups (2 k-chunks each)


def _pieces(lo, hi, step=512):
    """Split [lo, hi) at multiples of `step` (PSUM-bank-aligned chunks)."""
    out = []
    while lo < hi:
        nxt = min(hi, (lo // step + 1) * step)
        out.append((lo, nxt))
        lo = nxt
    return out


def build_nc(loop_n=1, phases=3):
    nc = bacc.Bacc("TRN2", target_bir_lowering=False, debug=False,
                   num_devices=NCORES)

    hT = nc.dram_tensor("hT", [HID, T], F32R, kind="ExternalInput").ap()
    wqkvT = nc.dram_tensor("wqkvT", [HID, NB * D], F32R,
                           kind="ExternalInput").ap()
    woT = nc.dram_tensor("woT", [HPC * D, HID], F32R, kind="ExternalInput").ap()
    # cos2 = [cos; cos], sinm2 = [-sin; +sin] stacked along d (see host_inputs)
    cosT = nc.dram_tensor("cosT", [D, T], F32, kind="ExternalInput").ap()
    sinT = nc.dram_tensor("sinT", [D, T], F32, kind="ExternalInput").ap()
    trim = nc.dram_tensor("trim", [128, 128], BF16, kind="ExternalInput").ap()
    onesm = nc.dram_tensor("onesm", [128, 128], FP16, kind="ExternalInput").ap()
    out = nc.dram_tensor("out", [T, HID], F32, kind="ExternalOutput").ap()

    import contextlib

    with tile.TileContext(nc) as tc, contextlib.ExitStack() as _loopctx:
        if loop_n > 1:
            _loopctx.enter_context(tc.For_i(0, loop_n))

        with contextlib.ExitStack() as _resctx:
            rpool = _resctx.enter_context(tc.tile_pool(name="res", bufs=1))
            # SBUF residents spanning phases: roped q heads [d, t], roped k
            # [d, t], v in block-column layout ([s_local, d] per 128-block).
            q_res = [rpool.tile([D, T], F32R, name=f"qres{h}", tag=f"qres{h}")
                     for h in range(HPC)]
            k_res = rpool.tile([D, T], F32R)
            v_res = rpool.tile([128, T], BF16)
            tri = rpool.tile([128, 128], BF16)
            nc.sync.dma_start(tri, trim)
            ones = rpool.tile([128, 128], FP16)
            nc.sync.dma_start(ones, onesm)
            ident = rpool.tile([128, 128], F32)
            make_identity(nc, ident)
            # half-swap permutation (swapm[d, i] = 1 iff |d - i| == 64),
            # assembled from identity blocks; used as a matmul stationary to
            # swap d-halves on the PE instead of an SBUF->SBUF DMA
            swapm = rpool.tile([128, 128], F32R)
            nc.vector.memset(swapm.bitcast(F32), 0.0)
            nc.sync.dma_start(swapm[0:64, 64:128],
                              ident.bitcast(F32R)[0:64, 0:64])
            nc.sync.dma_start(swapm[64:128, 0:64],
                              ident.bitcast(F32R)[64:128, 64:128])

            # ------------- Phase 1: QKV projection + RoPE + V transpose ----
            with tc.tile_pool(name="wq", bufs=1) as wpool, \
                 tc.tile_pool(name="hid", bufs=1) as hpool, \
                 tc.tile_pool(name="cs", bufs=2) as cspool, \
                 tc.tile_pool(name="stage", bufs=2) as stpool, \
                 tc.tile_pool(name="swap", bufs=2) as swpool, \
                 tc.tile_pool(name="vstage", bufs=2) as vspool, \
                 tc.tile_pool(name="qkvpsum", bufs=1, space="PSUM") as qpsum, \
                 tc.tile_pool(name="xps", bufs=1, space="PSUM") as xpsum:

                # weights: 8 group tiles [128, 4x768], one DMA each
                KPG = KCH // KG          # k-chunks per group = 4
                GW = NB * D * KPG        # columns per group tile
                wq = []
                for g in range(KG):
                    wt = wpool.tile([128, GW], F32R, name=f"wqg{g}",
                                    tag=f"wqg{g}")
                    wq.append(wt)

                def wq_dma(g):
                    dst = wq[g].rearrange("p (k n) -> p k n", k=KPG)
                    src = wqkvT[g * KPG * 128:(g + 1) * KPG * 128, :] \
                        .rearrange("(k p) n -> p k n", p=128)
                    nc.sync.dma_start(dst, src)

                def wq_sl(k, nb):
                    o = (k % KPG) * NB * D + nb * D
                    return wq[k // KPG][:, o:o + D]

                def chunk_inputs(tq):
                    tlo = tq * TQ
                    hq = []
                    for qd in range(8):
                        ht = hpool.tile([128, 4 * TQ], F32R, name=f"hq{qd}",
                                        tag=f"hq{qd}")
                        dst = ht.rearrange("p (k t) -> p k t", k=4)
                        src = hT[qd * 512:(qd + 1) * 512, tlo:tlo + TQ] \
                            .rearrange("(k p) t -> p k t", p=128)
                        nc.sync.dma_start(dst, src)
                        hq.append(ht)
                    cos_t = cspool.tile([128, TQ], F32, name="cos", tag="cos")
                    nc.sync.dma_start(cos_t, cosT[:, tlo:tlo + TQ])
                    sin_t = cspool.tile([128, TQ], F32, name="sin", tag="sin")
                    nc.sync.dma_start(sin_t, sinT[:, tlo:tlo + TQ])
                    return hq, cos_t, sin_t

                # issue order: first weight group, chunk-0 inputs, the rest
                # of the weights (so chunk-0 compute streams with the load)
                wq_dma(0)
                wq_dma(1)
                nxt = chunk_inputs(0)
                for g in range(2, KG):
                    wq_dma(g)

                for tq in range(NTQ):
                    tlo = tq * TQ
                    hq, cos_t, sin_t = nxt
                    if tq + 1 < NTQ:
                        nxt = chunk_inputs(tq + 1)

                    # one PSUM bank per output block (accumulation groups
                    # must be bank-exclusive)
                    pss = [qpsum.tile([128, TQ], F32, name=f"qkv_ps{nb}",
                                      tag=f"qkv_ps{nb}") for nb in range(NB)]

                    for k in range(KCH):
                        hsl = hq[k // 4][:, (k % 4) * TQ:(k % 4 + 1) * TQ]
                        for nb in range(NB):
                            nc.tensor.matmul(
                                pss[nb], wq_sl(k, nb), hsl,
                                start=(k == 0), stop=(k == KCH - 1))

                    raw5 = stpool.tile([128, NQK * TQ], F32R, name="raw5",
                                       tag="raw5")
                    # RoPE per block: drain, swap d-halves via permutation
                    # matmul on PE (DVE lanes are partition-fixed; this
                    # replaces an SBUF->SBUF DMA on the critical cadence)
                    for b in range(NQK):
                        bs = slice(b * TQ, (b + 1) * TQ)
                        nc.scalar.copy(raw5[:, bs], pss[b])
                        swp = xpsum.tile([128, TQ], F32, name="sw_ps",
                                         tag="sw_ps")
                        nc.tensor.matmul(swp, swapm, raw5[:, bs],
                                         start=True, stop=True)
                        swt = swpool.tile([128, TQ], F32, name="swt",
                                          tag="swt")
                        nc.vector.tensor_mul(swt, swp, sin_t)
                        nc.vector.tensor_mul(raw5[:, bs], raw5[:, bs], cos_t)
                        dst = q_res[b] if b < HPC else k_res
                        nc.vector.tensor_add(dst[:, tlo:tlo + TQ],
                                             raw5[:, bs], swt)
                    # V: drain, transpose 128x128 tiles on PE, store [s, d]
                    vs = vspool.tile([128, TQ], F32, name="vs", tag="vs")
                    nc.scalar.copy(vs, pss[NQK])
                    for i in range(TQ // 128):
                        tp = xpsum.tile([128, 128], F32, name="vt_ps",
                                        tag="vt_ps")
                        nc.tensor.transpose(tp, vs[:, i * 128:(i + 1) * 128],
                                            ident)
                        j = 2 * tq + i
                        nc.vector.tensor_copy(v_res[:, j * 128:(j + 1) * 128],
                                              tp)

            # ---------------- Phase 2: attention per head ------------------
            if phases >= 2:
                opool = _resctx.enter_context(
                    tc.tile_pool(name="attn_out", bufs=1))
                wopool = _resctx.enter_context(
                    tc.tile_pool(name="wo", bufs=1))
                wo = []
                for h in range(HPC):
                    wt = wopool.tile([128, HID], F32R, name=f"wo{h}",
                                     tag=f"wo{h}")
                    nc.sync.dma_start(wt, woT[h * D:(h + 1) * D, :])
                    wo.append(wt)
                attn = []
                for h in range(HPC):
                    at = opool.tile([D, T], F32R, name=f"attn{h}",
                                    tag=f"attn{h}")
                    attn.append(at)

                with tc.tile_pool(name="pj", bufs=1) as ppool, \
                     tc.tile_pool(name="dna", bufs=2) as dpool, \
                     tc.tile_pool(name="rec", bufs=2) as rpool2, \
                     tc.tile_pool(name="scps", bufs=2, space="PSUM") as scps, \
                     tc.tile_pool(name="pvps", bufs=2, space="PSUM") as pvps:

                    # The two causal halves of a head are independent
                    # dependency chains; interleaving them (2 units of half1
                    # per unit of half0) keeps PE/Act/DVE all fed despite the
                    # serial sc -> exp -> mask -> pv chain within each unit.
                    for h in range(HPC):
                        qt = q_res[h]

                        class _Stream:
                            pass

                        def make_stream(s):
                            st = _Stream()
                            st.s = s
                            st.q_lo = 1024 * s
                            st.pv = pvps.tile([128, 1024], F32,
                                              name=f"pv{s}", tag="pv")
                            st.dna = dpool.tile([128, 1024], FP16,
                                                name=f"dna{s}", tag=f"dna{s}")
                            st.prev = None
                            st.pj_all = []
                            return st

                        def emit_pv(st, j, le, pj):
                            # far-from-diagonal pieces first: they only
                            # depend on exp, not on the tri-mask mul
                            for (plo, phi) in reversed(_pieces(le, 1024)):
                                last = (st.q_lo + phi) // 128 - 1
                                nc.tensor.matmul(
                                    st.pv[:, plo:phi],
                                    v_res[:, j * 128:(j + 1) * 128],
                                    pj[:, plo - le:phi - le],
                                    start=(j == 0), stop=(j == last))

                        def emit_unit(st, j):
                            ls = max(0, 128 * j - st.q_lo)
                            # extend so every matmul piece is >=256 wide
                            # (narrow fp32r matmuls run at 1/4 rate); the
                            # extension columns are zeroed after exp.
                            le = ls - 128 if ls % 512 == 384 else ls
                            pj = ppool.tile([128, 1024 - le], BF16,
                                            name=f"p{st.s}_{j}",
                                            tag=f"p{st.s}_{j}")
                            sc = scps.tile([128, 1024], F32,
                                           name=f"sc{st.s}", tag="sc")
                            for (plo, phi) in _pieces(le, 1024):
                                nc.tensor.matmul(
                                    sc[:, plo:phi],
                                    k_res[:, j * 128:(j + 1) * 128],
                                    qt[:, st.q_lo + plo:st.q_lo + phi],
                                    start=True, stop=True)
                            nc.scalar.activation(
                                pj, sc[:, le:1024],
                                mybir.ActivationFunctionType.Exp,
                                scale=SCALE)
                            if le < ls:
                                nc.vector.memset(pj[:, 0:ls - le], 0.0)
                            if 128 * j >= st.q_lo:
                                o = ls - le
                                nc.vector.tensor_mul(
                                    pj[:, o:o + 128], pj[:, o:o + 128], tri)
                            # software pipeline: PV of the previous j comes
                            # after this j's score matmuls, so PE always has
                            # mask-independent work queued while the
                            # Activation engine runs exp.
                            if st.prev is not None:
                                emit_pv(st, *st.prev)
                            st.prev = (j, le, pj)
                            st.pj_all.append((le, pj))

                        def finish_stream(st):
                            emit_pv(st, *st.prev)
                            # denominator column-sums batched at stream end
                            # so they stay off the per-j critical cadence
                            for i, (le, pj) in enumerate(st.pj_all):
                                if i == 0:
                                    nc.vector.tensor_copy(st.dna, pj)
                                else:
                                    nc.vector.tensor_add(
                                        st.dna[:, le:1024],
                                        st.dna[:, le:1024], pj)
                            dn = scps.tile([128, 1024], F32, name=f"dn{st.s}",
                                           tag="sc")
                            for (plo, phi) in _pieces(0, 1024):
                                nc.tensor.matmul(dn[:, plo:phi], ones,
                                                 st.dna[:, plo:phi],
                                                 start=True, stop=True)
                            rec = rpool2.tile([128, 1024], F32,
                                              name=f"rec{st.s}",
                                              tag=f"rec{st.s}")
                            nc.vector.reciprocal(rec, dn)
                            nc.vector.tensor_mul(
                                attn[h][:, st.q_lo:st.q_lo + 1024],
                                st.pv, rec)

                        for s in range(2):
                            st = make_stream(s)
                            for j in range(8 * (s + 1)):
                                emit_unit(st, j)
                            finish_stream(st)

                # ---------------- Phase 3: o_proj partial ------------------
                if phases >= 3:
                    with tc.tile_pool(name="ostage", bufs=3) as ospool, \
                         tc.tile_pool(name="ops", bufs=2,
                                      space="PSUM") as opsum:
                        for tb in range(T // 128):
                            for ch in range(2):
                                ps = opsum.tile([128, 2048], F32, name="o_ps",
                                                tag="o_ps")
                                for h in range(HPC):
                                    lhs = attn[h][:, tb * 128:(tb + 1) * 128]
                                    for cc in range(4):
                                        cl = ch * 2048 + cc * 512
                                        nc.tensor.matmul(
                                            ps[:, cc * 512:(cc + 1) * 512],
                                            lhs, wo[h][:, cl:cl + 512],
                                            start=(h == 0),
                                            stop=(h == HPC - 1))
                                ob = ospool.tile([128, 2048], F32, name="ob",
                                                 tag="ob")
                                orow = out[tb * 128:(tb + 1) * 128,
                                           ch * 2048:(ch + 1) * 2048]
                                if tb == T // 128 - 1 and ch == 1:
                                    # pipeline the drain of the very last
                                    # tile so the kernel tail is short
                                    for q4 in range(4):
                                        sl = slice(q4 * 512, (q4 + 1) * 512)
                                        nc.scalar.copy(ob[:, sl], ps[:, sl])
                                        nc.sync.dma_start(orow[:, sl],
                                                          ob[:, sl])
                                else:
                                    nc.scalar.copy(ob, ps)
                                    for q2 in range(2):
                                        sl = slice(q2 * 1024, (q2 + 1) * 1024)
                                        nc.sync.dma_start(orow[:, sl],
                                                          ob[:, sl])
    nc.compile()
    return nc


def host_inputs(hidden_states, positions, Wqkv, Wo):
    """Build the 8 per-core input maps (host-side sharding + layout prep)."""
    f = np.float32
    hT = np.ascontiguousarray(hidden_states.T.astype(f))
    half = D // 2
    inv_freq = 1.0 / (THETA ** (np.arange(half, dtype=np.float64) / half))
    ang = inv_freq[:, None] * positions.astype(np.float64)[None, :]
    cos = np.cos(ang).astype(f)
    sin = np.sin(ang).astype(f)
    cosT = np.vstack([cos, cos])                  # [D, T]
    sinT = np.vstack([-sin, sin])                 # rotate-half sign baked in
    trim = (np.arange(128)[:, None] <= np.arange(128)[None, :]) \
        .astype(ml_dtypes.bfloat16)
    onesm = np.ones((128, 128), np.float16)

    in_maps = []
    for c in range(NCORES):
        rows = list(range(c * HPC * D, (c + 1) * HPC * D))          # Q heads
        rows += list(range(H * D + c * D, H * D + (c + 1) * D))     # K head
        rows += list(range((H + KVH) * D + c * D,
                           (H + KVH) * D + (c + 1) * D))            # V head
        wqkvT = np.ascontiguousarray(Wqkv[rows, :].T.astype(f))
        woT = np.ascontiguousarray(Wo[:, c * HPC * D:(c + 1) * HPC * D].T
                                   .astype(f))
        in_maps.append({"hT": hT, "wqkvT": wqkvT, "woT": woT,
                        "cosT": cosT, "sinT": sinT, "trim": trim,
                        "onesm": onesm})
    return in_maps


_NC_CACHE = {}


def get_nc(loop_n=1, phases=3):
    key = (loop_n, phases)
    if key not in _NC_CACHE:
        _NC_CACHE[key] = build_nc(loop_n, phases)
    return _NC_CACHE[key]


def kernel(hidden_states, positions, Wqkv, Wo, _trace=False):
    hidden_states = np.asarray(hidden_states)
    positions = np.asarray(positions)
    Wqkv = np.asarray(Wqkv)
    Wo = np.asarray(Wo)
    in_maps = host_inputs(hidden_states, positions, Wqkv, Wo)
    nc = get_nc()
    res = bass_utils.run_bass_kernel_spmd(
        nc, in_maps, core_ids=list(range(NCORES)), trace=_trace)
    acc = np.zeros((T, HID), np.float64)
    for r in res.results:
        acc += r["out"].astype(np.float64)
    out = acc.astype(np.float32)
    if _trace:
        return out, res
    return out


# revision 27
# speedup vs baseline: 276.7783x; 1.0335x over previous
"""Trainium2 Bass kernel for Llama-style GQA attention (T=2048, HID=4096,
H=32 q-heads, KV=8 kv-heads, D=128), tensor-parallel over heads on 8 cores.

Per-core work (core c):
  - QKV projection for its 4 q-heads + 1 kv-head (K and V), k-major over
    the contraction so the streamed h tiles free early; RoPE fused into
    the PSUM drains.  Roped q/k (fp32r) and PE-transposed V (bf16) stay
    SBUF-resident -- no DRAM scratch roundtrip.  (An xbar dma_transpose
    for V passed CoreSim but produced wrong data on hardware; PE
    transpose is the validated path.)
  - Causal attention per head as scores^T [s, q]: per s-block j the
    score matmul, exp (Activation), causal mask + denominator
    accumulation (DVE, fp16), and PV matmul (PE) pipeline so the
    Activation-bound exp overlaps the PE work.  Softmax skips the
    max-subtraction (scores are O(10), exp is safe in fp32); the
    denominator is a ones-stationary matmul over the DVE-accumulated
    column sums.  P/V are bf16 (same PE rate, half the SBUF cost).
  - Partial o_proj: attn^T(local heads) x Wo^T(local rows) -> [T, HID]
    partial sum.  Host adds the 8 partials (the "all-reduce").

DMA discipline: every dma_start costs ~0.6us on the serial HWDGE +
sequencer, so transfers are batched into few multi-dim descriptors
(~80 total).  Weight/h loads are interleaved in issue order so the first
QKV matmuls are not stuck behind the full 12.6MB weight transfer.
"""

import numpy as np
import ml_dtypes

import concourse.bass as bass
import concourse.bacc as bacc
import concourse.mybir as mybir
import concourse.tile as tile
from concourse import bass_utils
from concourse.masks import make_identity

T = 2048
HID = 4096
H = 32
KVH = 8
D = 128
NCORES = 8
HPC = H // NCORES          # q-heads per core = 4
THETA = 10000.0
F32 = mybir.dt.float32
F32R = mybir.dt.float32r
BF16 = mybir.dt.bfloat16
FP16 = mybir.dt.float16
SCALE = float(D) ** -0.5

# QKV projection output blocks per core: 4 q-heads, 1 k-head, 1 v-head
NB = HPC + 2               # 6 blocks of 128
NQK = HPC + 1              # blocks 0..4 get RoPE (Q0..Q3, K); block 5 is V

TQ = 256                   # QKV t-chunk width (8 chunks)
NTQ = T // TQ
KCH = HID // 128           # 32 contraction chunks
KG = 16                    # wq DMA groups (2 k-chunks each)


def _pieces(lo, hi, step=512):
    """Split [lo, hi) at multiples of `step` (PSUM-bank-aligned chunks)."""
    out = []
    while lo < hi:
        nxt = min(hi, (lo // step + 1) * step)
        out.append((lo, nxt))
        lo = nxt
    return out


def build_nc(loop_n=1, phases=3):
    nc = bacc.Bacc("TRN2", target_bir_lowering=False, debug=False,
                   num_devices=NCORES)

    hT = nc.dram_tensor("hT", [HID, T], F32R, kind="ExternalInput").ap()
    wqkvT = nc.dram_tensor("wqkvT", [HID, NB * D], F32R,
                           kind="ExternalInput").ap()
    woT = nc.dram_tensor("woT", [HPC * D, HID], F32R, kind="ExternalInput").ap()
    # cos2 = [cos; cos], sinm2 = [-sin; +sin] stacked along d (see host_inputs)
    cosT = nc.dram_tensor("cosT", [D, T], F32, kind="ExternalInput").ap()
    sinT = nc.dram_tensor("sinT", [D, T], F32, kind="ExternalInput").ap()
    trim = nc.dram_tensor("trim", [128, 128], BF16, kind="ExternalInput").ap()
    onesm = nc.dram_tensor("onesm", [128, 128], FP16, kind="ExternalInput").ap()
    out = nc.dram_tensor("out", [T, HID], BF16,
                     kind="ExternalOutput").ap()

    import contextlib

    with tile.TileContext(nc) as tc, contextlib.ExitStack() as _loopctx:
        if loop_n > 1:
            _loopctx.enter_context(tc.For_i(0, loop_n))

        with contextlib.ExitStack() as _resctx:
            rpool = _resctx.enter_context(tc.tile_pool(name="res", bufs=1))
            # SBUF residents spanning phases: roped q heads [d, t], roped k
            # [d, t], v in block-column layout ([s_local, d] per 128-block).
            q_res = [rpool.tile([D, T], F32R, name=f"qres{h}", tag=f"qres{h}")
                     for h in range(HPC)]
            k_res = rpool.tile([D, T], F32R)
            v_res = rpool.tile([128, T], BF16)
            tri = rpool.tile([128, 128], BF16)
            nc.sync.dma_start(tri, trim)
            ones = rpool.tile([128, 128], FP16)
            nc.sync.dma_start(ones, onesm)
            ident = rpool.tile([128, 128], F32)
            make_identity(nc, ident)
            # half-swap permutation (swapm[d, i] = 1 iff |d - i| == 64),
            # assembled from identity blocks; used as a matmul stationary to
            # swap d-halves on the PE instead of an SBUF->SBUF DMA
            swapm = rpool.tile([128, 128], F32R)
            nc.vector.memset(swapm.bitcast(F32), 0.0)
            nc.sync.dma_start(swapm[0:64, 64:128],
                              ident.bitcast(F32R)[0:64, 0:64])
            nc.sync.dma_start(swapm[64:128, 0:64],
                              ident.bitcast(F32R)[64:128, 64:128])

            # ------------- Phase 1: QKV projection + RoPE + V transpose ----
            with tc.tile_pool(name="wq", bufs=1) as wpool, \
                 tc.tile_pool(name="hid", bufs=1) as hpool, \
                 tc.tile_pool(name="cs", bufs=2) as cspool, \
                 tc.tile_pool(name="stage", bufs=2) as stpool, \
                 tc.tile_pool(name="swap", bufs=2) as swpool, \
                 tc.tile_pool(name="vstage", bufs=2) as vspool, \
                 tc.tile_pool(name="qkvpsum", bufs=1, space="PSUM") as qpsum, \
                 tc.tile_pool(name="xps", bufs=1, space="PSUM") as xpsum:

                # weights: 8 group tiles [128, 4x768], one DMA each
                KPG = KCH // KG          # k-chunks per group = 4
                GW = NB * D * KPG        # columns per group tile
                wq = []
                for g in range(KG):
                    wt = wpool.tile([128, GW], F32R, name=f"wqg{g}",
                                    tag=f"wqg{g}")
                    wq.append(wt)

                def wq_dma(g):
                    dst = wq[g].rearrange("p (k n) -> p k n", k=KPG)
                    src = wqkvT[g * KPG * 128:(g + 1) * KPG * 128, :] \
                        .rearrange("(k p) n -> p k n", p=128)
                    nc.sync.dma_start(dst, src)

                def wq_sl(k, nb):
                    o = (k % KPG) * NB * D + nb * D
                    return wq[k // KPG][:, o:o + D]

                def chunk_inputs(tq):
                    tlo = tq * TQ
                    hq = []
                    for qd in range(16):
                        ht = hpool.tile([128, 2 * TQ], F32R, name=f"hq{qd}",
                                        tag=f"hq{qd}")
                        dst = ht.rearrange("p (k t) -> p k t", k=2)
                        src = hT[qd * 256:(qd + 1) * 256, tlo:tlo + TQ] \
                            .rearrange("(k p) t -> p k t", p=128)
                        nc.sync.dma_start(dst, src)
                        hq.append(ht)
                    cos_t = cspool.tile([128, TQ], F32, name="cos", tag="cos")
                    nc.sync.dma_start(cos_t, cosT[:, tlo:tlo + TQ])
                    sin_t = cspool.tile([128, TQ], F32, name="sin", tag="sin")
                    nc.sync.dma_start(sin_t, sinT[:, tlo:tlo + TQ])
                    return hq, cos_t, sin_t

                # issue order: first weight group, chunk-0 inputs, the rest
                # of the weights (so chunk-0 compute streams with the load)
                wq_dma(0)
                wq_dma(1)
                nxt = chunk_inputs(0)
                for g in range(2, KG):
                    wq_dma(g)

                for tq in range(NTQ):
                    tlo = tq * TQ
                    hq, cos_t, sin_t = nxt
                    if tq + 1 < NTQ:
                        nxt = chunk_inputs(tq + 1)

                    # one PSUM bank per output block (accumulation groups
                    # must be bank-exclusive)
                    pss = [qpsum.tile([128, TQ], F32, name=f"qkv_ps{nb}",
                                      tag=f"qkv_ps{nb}") for nb in range(NB)]

                    for k in range(KCH):
                        hsl = hq[k // 2][:, (k % 2) * TQ:(k % 2 + 1) * TQ]
                        for nb in range(NB):
                            nc.tensor.matmul(
                                pss[nb], wq_sl(k, nb), hsl,
                                start=(k == 0), stop=(k == KCH - 1))

                    raw5 = stpool.tile([128, NQK * TQ], F32R, name="raw5",
                                       tag="raw5")
                    # RoPE per block: drain, swap d-halves via permutation
                    # matmul on PE (DVE lanes are partition-fixed; this
                    # replaces an SBUF->SBUF DMA on the critical cadence)
                    for b in range(NQK):
                        bs = slice(b * TQ, (b + 1) * TQ)
                        nc.scalar.copy(raw5[:, bs], pss[b])
                        swp = xpsum.tile([128, TQ], F32, name="sw_ps",
                                         tag="sw_ps")
                        nc.tensor.matmul(swp, swapm, raw5[:, bs],
                                         start=True, stop=True)
                        swt = swpool.tile([128, TQ], F32, name="swt",
                                          tag="swt")
                        nc.vector.tensor_mul(swt, swp, sin_t)
                        nc.vector.tensor_mul(raw5[:, bs], raw5[:, bs], cos_t)
                        dst = q_res[b] if b < HPC else k_res
                        nc.vector.tensor_add(dst[:, tlo:tlo + TQ],
                                             raw5[:, bs], swt)
                    # V: drain, transpose 128x128 tiles on PE, store [s, d]
                    vs = vspool.tile([128, TQ], F32, name="vs", tag="vs")
                    nc.scalar.copy(vs, pss[NQK])
                    for i in range(TQ // 128):
                        tp = xpsum.tile([128, 128], F32, name="vt_ps",
                                        tag="vt_ps")
                        nc.tensor.transpose(tp, vs[:, i * 128:(i + 1) * 128],
                                            ident)
                        j = 2 * tq + i
                        nc.vector.tensor_copy(v_res[:, j * 128:(j + 1) * 128],
                                              tp)

            # ---------------- Phase 2: attention per head ------------------
            if phases >= 2:
                opool = _resctx.enter_context(
                    tc.tile_pool(name="attn_out", bufs=1))
                wopool = _resctx.enter_context(
                    tc.tile_pool(name="wo", bufs=1))
                wo = []
                for h in range(HPC):
                    wt = wopool.tile([128, HID], F32R, name=f"wo{h}",
                                     tag=f"wo{h}")
                    for wp in range(4):
                        sl = slice(wp * 1024, (wp + 1) * 1024)
                        nc.sync.dma_start(wt[:, sl],
                                          woT[h * D:(h + 1) * D, sl])
                    wo.append(wt)
                attn = []
                for h in range(HPC):
                    at = opool.tile([D, T], F32R, name=f"attn{h}",
                                    tag=f"attn{h}")
                    attn.append(at)

                with tc.tile_pool(name="pj", bufs=1) as ppool, \
                     tc.tile_pool(name="dna", bufs=2) as dpool, \
                     tc.tile_pool(name="rec", bufs=2) as rpool2, \
                     tc.tile_pool(name="scps", bufs=2, space="PSUM") as scps, \
                     tc.tile_pool(name="pvps", bufs=2, space="PSUM") as pvps:

                    # The two causal halves of a head are independent
                    # dependency chains; interleaving them (2 units of half1
                    # per unit of half0) keeps PE/Act/DVE all fed despite the
                    # serial sc -> exp -> mask -> pv chain within each unit.
                    for h in range(HPC):
                        qt = q_res[h]

                        class _Stream:
                            pass

                        def make_stream(s):
                            st = _Stream()
                            st.s = s
                            st.q_lo = 1024 * s
                            st.pv = pvps.tile([128, 1024], F32,
                                              name=f"pv{s}", tag="pv")
                            st.dna = dpool.tile([128, 1024], FP16,
                                                name=f"dna{s}", tag=f"dna{s}")
                            st.prev = None
                            st.pj_all = []
                            return st

                        def emit_pv(st, j, le, pj):
                            # far-from-diagonal pieces first: they only
                            # depend on exp, not on the tri-mask mul
                            for (plo, phi) in reversed(_pieces(le, 1024)):
                                last = (st.q_lo + phi) // 128 - 1
                                nc.tensor.matmul(
                                    st.pv[:, plo:phi],
                                    v_res[:, j * 128:(j + 1) * 128],
                                    pj[:, plo - le:phi - le],
                                    start=(j == 0), stop=(j == last))

                        def emit_unit(st, j):
                            ls = max(0, 128 * j - st.q_lo)
                            # extend so every matmul piece is >=256 wide
                            # (narrow fp32r matmuls run at 1/4 rate); the
                            # extension columns are zeroed after exp.
                            le = ls - 128 if ls % 512 == 384 else ls
                            pj = ppool.tile([128, 1024 - le], BF16,
                                            name=f"p{st.s}_{j}",
                                            tag=f"p{st.s}_{j}")
                            sc = scps.tile([128, 1024], F32,
                                           name=f"sc{st.s}", tag="sc")
                            for (plo, phi) in _pieces(le, 1024):
                                nc.tensor.matmul(
                                    sc[:, plo:phi],
                                    k_res[:, j * 128:(j + 1) * 128],
                                    qt[:, st.q_lo + plo:st.q_lo + phi],
                                    start=True, stop=True)
                            nc.scalar.activation(
                                pj, sc[:, le:1024],
                                mybir.ActivationFunctionType.Exp,
                                scale=SCALE)
                            if le < ls:
                                nc.vector.memset(pj[:, 0:ls - le], 0.0)
                            if 128 * j >= st.q_lo:
                                o = ls - le
                                nc.vector.tensor_mul(
                                    pj[:, o:o + 128], pj[:, o:o + 128], tri)
                            # software pipeline: PV of the previous j comes
                            # after this j's score matmuls, so PE always has
                            # mask-independent work queued while the
                            # Activation engine runs exp.
                            if st.prev is not None:
                                emit_pv(st, *st.prev)
                            st.prev = (j, le, pj)
                            st.pj_all.append((le, pj))

                        def finish_stream(st):
                            emit_pv(st, *st.prev)
                            # denominator column-sums batched at stream end
                            # so they stay off the per-j critical cadence
                            for i, (le, pj) in enumerate(st.pj_all):
                                if i == 0:
                                    nc.vector.tensor_copy(st.dna, pj)
                                else:
                                    nc.vector.tensor_add(
                                        st.dna[:, le:1024],
                                        st.dna[:, le:1024], pj)
                            dn = scps.tile([128, 1024], F32, name=f"dn{st.s}",
                                           tag="sc")
                            for (plo, phi) in _pieces(0, 1024):
                                nc.tensor.matmul(dn[:, plo:phi], ones,
                                                 st.dna[:, plo:phi],
                                                 start=True, stop=True)
                            rec = rpool2.tile([128, 1024], F32,
                                              name=f"rec{st.s}",
                                              tag=f"rec{st.s}")
                            nc.vector.reciprocal(rec, dn)
                            nc.vector.tensor_mul(
                                attn[h][:, st.q_lo:st.q_lo + 1024],
                                st.pv, rec)

                        for s in range(2):
                            st = make_stream(s)
                            for j in range(8 * (s + 1)):
                                emit_unit(st, j)
                            finish_stream(st)

                # ---------------- Phase 3: o_proj partial ------------------
                if phases >= 3:
                    with tc.tile_pool(name="ostage", bufs=3) as ospool, \
                         tc.tile_pool(name="ops", bufs=2,
                                      space="PSUM") as opsum:
                        for tb in range(T // 128):
                            for ch in range(2):
                                ps = opsum.tile([128, 2048], F32, name="o_ps",
                                                tag="o_ps")
                                for h in range(HPC):
                                    lhs = attn[h][:, tb * 128:(tb + 1) * 128]
                                    for cc in range(4):
                                        cl = ch * 2048 + cc * 512
                                        nc.tensor.matmul(
                                            ps[:, cc * 512:(cc + 1) * 512],
                                            lhs, wo[h][:, cl:cl + 512],
                                            start=(h == 0),
                                            stop=(h == HPC - 1))
                                ob = ospool.tile([128, 2048], BF16, name="ob",
                                                 tag="ob")
                                orow = out[tb * 128:(tb + 1) * 128,
                                           ch * 2048:(ch + 1) * 2048]
                                if tb == T // 128 - 1 and ch == 1:
                                    # pipeline the drain of the very last
                                    # tile so the kernel tail is short
                                    for q4 in range(4):
                                        sl = slice(q4 * 512, (q4 + 1) * 512)
                                        nc.scalar.copy(ob[:, sl], ps[:, sl])
                                        nc.sync.dma_start(orow[:, sl],
                                                          ob[:, sl])
                                else:
                                    nc.scalar.copy(ob, ps)
                                    for q2 in range(2):
                                        sl = slice(q2 * 1024, (q2 + 1) * 1024)
                                        nc.sync.dma_start(orow[:, sl],
                                                          ob[:, sl])
    nc.compile()
    return nc


def host_inputs(hidden_states, positions, Wqkv, Wo):
    """Build the 8 per-core input maps (host-side sharding + layout prep)."""
    f = np.float32
    hT = np.ascontiguousarray(hidden_states.T.astype(f))
    half = D // 2
    inv_freq = 1.0 / (THETA ** (np.arange(half, dtype=np.float64) / half))
    ang = inv_freq[:, None] * positions.astype(np.float64)[None, :]
    cos = np.cos(ang).astype(f)
    sin = np.sin(ang).astype(f)
    cosT = np.vstack([cos, cos])                  # [D, T]
    sinT = np.vstack([-sin, sin])                 # rotate-half sign baked in
    trim = (np.arange(128)[:, None] <= np.arange(128)[None, :]) \
        .astype(ml_dtypes.bfloat16)
    onesm = np.ones((128, 128), np.float16)

    in_maps = []
    for c in range(NCORES):
        rows = list(range(c * HPC * D, (c + 1) * HPC * D))          # Q heads
        rows += list(range(H * D + c * D, H * D + (c + 1) * D))     # K head
        rows += list(range((H + KVH) * D + c * D,
                           (H + KVH) * D + (c + 1) * D))            # V head
        wqkvT = np.ascontiguousarray(Wqkv[rows, :].T.astype(f))
        woT = np.ascontiguousarray(Wo[:, c * HPC * D:(c + 1) * HPC * D].T
                                   .astype(f))
        in_maps.append({"hT": hT, "wqkvT": wqkvT, "woT": woT,
                        "cosT": cosT, "sinT": sinT, "trim": trim,
                        "onesm": onesm})
    return in_maps


_NC_CACHE = {}


def get_nc(loop_n=1, phases=3):
    key = (loop_n, phases)
    if key not in _NC_CACHE:
        _NC_CACHE[key] = build_nc(loop_n, phases)
    return _NC_CACHE[key]


def kernel(hidden_states, positions, Wqkv, Wo, _trace=False):
    hidden_states = np.asarray(hidden_states)
    positions = np.asarray(positions)
    Wqkv = np.asarray(Wqkv)
    Wo = np.asarray(Wo)
    in_maps = host_inputs(hidden_states, positions, Wqkv, Wo)
    nc = get_nc()
    res = bass_utils.run_bass_kernel_spmd(
        nc, in_maps, core_ids=list(range(NCORES)), trace=_trace)
    acc = np.zeros((T, HID), np.float64)
    for r in res.results:
        acc += r["out"].astype(np.float64)
    out = acc.astype(np.float32)
    if _trace:
        return out, res
    return out


# revision 28
# speedup vs baseline: 282.3533x; 1.0201x over previous
"""Trainium2 Bass kernel for Llama-style GQA attention (T=2048, HID=4096,
H=32 q-heads, KV=8 kv-heads, D=128), tensor-parallel over heads on 8 cores.

Per-core work (core c):
  - QKV projection for its 4 q-heads + 1 kv-head (K and V), k-major over
    the contraction so the streamed h tiles free early; RoPE fused into
    the PSUM drains.  Roped q/k (fp32r) and PE-transposed V (bf16) stay
    SBUF-resident -- no DRAM scratch roundtrip.  (An xbar dma_transpose
    for V passed CoreSim but produced wrong data on hardware; PE
    transpose is the validated path.)
  - Causal attention per head as scores^T [s, q]: per s-block j the
    score matmul, exp (Activation), causal mask + denominator
    accumulation (DVE, fp16), and PV matmul (PE) pipeline so the
    Activation-bound exp overlaps the PE work.  Softmax skips the
    max-subtraction (scores are O(10), exp is safe in fp32); the
    denominator is a ones-stationary matmul over the DVE-accumulated
    column sums.  P/V are bf16 (same PE rate, half the SBUF cost).
  - Partial o_proj: attn^T(local heads) x Wo^T(local rows) -> [T, HID]
    partial sum.  Host adds the 8 partials (the "all-reduce").

DMA discipline: every dma_start costs ~0.6us on the serial HWDGE +
sequencer, so transfers are batched into few multi-dim descriptors
(~80 total).  Weight/h loads are interleaved in issue order so the first
QKV matmuls are not stuck behind the full 12.6MB weight transfer.
"""

import numpy as np
import ml_dtypes

import concourse.bass as bass
import concourse.bacc as bacc
import concourse.mybir as mybir
import concourse.tile as tile
from concourse import bass_utils
from concourse.masks import make_identity

T = 2048
HID = 4096
H = 32
KVH = 8
D = 128
NCORES = 8
HPC = H // NCORES          # q-heads per core = 4
THETA = 10000.0
F32 = mybir.dt.float32
F32R = mybir.dt.float32r
BF16 = mybir.dt.bfloat16
FP16 = mybir.dt.float16
SCALE = float(D) ** -0.5

# QKV projection output blocks per core: 4 q-heads, 1 k-head, 1 v-head
NB = HPC + 2               # 6 blocks of 128
NQK = HPC + 1              # blocks 0..4 get RoPE (Q0..Q3, K); block 5 is V

TQ = 256                   # QKV t-chunk width (8 chunks)
NTQ = T // TQ
KCH = HID // 128           # 32 contraction chunks
KG = 16                    # wq DMA groups (2 k-chunks each)


def _pieces(lo, hi, step=512):
    """Split [lo, hi) at multiples of `step` (PSUM-bank-aligned chunks)."""
    out = []
    while lo < hi:
        nxt = min(hi, (lo // step + 1) * step)
        out.append((lo, nxt))
        lo = nxt
    return out


def build_nc(loop_n=1, phases=3):
    nc = bacc.Bacc("TRN2", target_bir_lowering=False, debug=False,
                   num_devices=NCORES)

    hT = nc.dram_tensor("hT", [HID, T], BF16, kind="ExternalInput").ap()
    wqkvT = nc.dram_tensor("wqkvT", [HID, NB * D], BF16,
                           kind="ExternalInput").ap()
    woT = nc.dram_tensor("woT", [HPC * D, HID], F32R, kind="ExternalInput").ap()
    # cos2 = [cos; cos], sinm2 = [-sin; +sin] stacked along d (see host_inputs)
    cosT = nc.dram_tensor("cosT", [D, T], F32, kind="ExternalInput").ap()
    sinT = nc.dram_tensor("sinT", [D, T], F32, kind="ExternalInput").ap()
    trim = nc.dram_tensor("trim", [128, 128], BF16, kind="ExternalInput").ap()
    onesm = nc.dram_tensor("onesm", [128, 128], FP16, kind="ExternalInput").ap()
    out = nc.dram_tensor("out", [T, HID], BF16,
                     kind="ExternalOutput").ap()

    import contextlib

    with tile.TileContext(nc) as tc, contextlib.ExitStack() as _loopctx:
        if loop_n > 1:
            _loopctx.enter_context(tc.For_i(0, loop_n))

        with contextlib.ExitStack() as _resctx:
            rpool = _resctx.enter_context(tc.tile_pool(name="res", bufs=1))
            # SBUF residents spanning phases: roped q heads [d, t], roped k
            # [d, t], v in block-column layout ([s_local, d] per 128-block).
            q_res = [rpool.tile([D, T], F32R, name=f"qres{h}", tag=f"qres{h}")
                     for h in range(HPC)]
            k_res = rpool.tile([D, T], F32R)
            v_res = rpool.tile([128, T], BF16)
            tri = rpool.tile([128, 128], BF16)
            nc.sync.dma_start(tri, trim)
            ones = rpool.tile([128, 128], FP16)
            nc.sync.dma_start(ones, onesm)
            ident = rpool.tile([128, 128], F32)
            make_identity(nc, ident)
            # half-swap permutation (swapm[d, i] = 1 iff |d - i| == 64),
            # assembled from identity blocks; used as a matmul stationary to
            # swap d-halves on the PE instead of an SBUF->SBUF DMA
            swapm = rpool.tile([128, 128], F32R)
            nc.vector.memset(swapm.bitcast(F32), 0.0)
            nc.sync.dma_start(swapm[0:64, 64:128],
                              ident.bitcast(F32R)[0:64, 0:64])
            nc.sync.dma_start(swapm[64:128, 0:64],
                              ident.bitcast(F32R)[64:128, 64:128])

            # ------------- Phase 1: QKV projection + RoPE + V transpose ----
            with tc.tile_pool(name="wq", bufs=1) as wpool, \
                 tc.tile_pool(name="hid", bufs=1) as hpool, \
                 tc.tile_pool(name="cs", bufs=2) as cspool, \
                 tc.tile_pool(name="stage", bufs=2) as stpool, \
                 tc.tile_pool(name="swap", bufs=2) as swpool, \
                 tc.tile_pool(name="vstage", bufs=2) as vspool, \
                 tc.tile_pool(name="qkvpsum", bufs=1, space="PSUM") as qpsum, \
                 tc.tile_pool(name="xps", bufs=1, space="PSUM") as xpsum:

                # weights: 8 group tiles [128, 4x768], one DMA each
                KPG = KCH // KG          # k-chunks per group = 4
                GW = NB * D * KPG        # columns per group tile
                wq = []
                for g in range(KG):
                    wt = wpool.tile([128, GW], BF16, name=f"wqg{g}",
                                    tag=f"wqg{g}")
                    wq.append(wt)

                def wq_dma(g):
                    dst = wq[g].rearrange("p (k n) -> p k n", k=KPG)
                    src = wqkvT[g * KPG * 128:(g + 1) * KPG * 128, :] \
                        .rearrange("(k p) n -> p k n", p=128)
                    nc.sync.dma_start(dst, src)

                def wq_sl(k, nb):
                    o = (k % KPG) * NB * D + nb * D
                    return wq[k // KPG][:, o:o + D]

                def chunk_inputs(tq):
                    tlo = tq * TQ
                    hq = []
                    for qd in range(8):
                        ht = hpool.tile([128, 4 * TQ], BF16, name=f"hq{qd}",
                                        tag=f"hq{qd}")
                        dst = ht.rearrange("p (k t) -> p k t", k=4)
                        src = hT[qd * 512:(qd + 1) * 512, tlo:tlo + TQ] \
                            .rearrange("(k p) t -> p k t", p=128)
                        nc.sync.dma_start(dst, src)
                        hq.append(ht)
                    cos_t = cspool.tile([128, TQ], F32, name="cos", tag="cos")
                    nc.sync.dma_start(cos_t, cosT[:, tlo:tlo + TQ])
                    sin_t = cspool.tile([128, TQ], F32, name="sin", tag="sin")
                    nc.sync.dma_start(sin_t, sinT[:, tlo:tlo + TQ])
                    return hq, cos_t, sin_t

                # issue order: first weight group, chunk-0 inputs, the rest
                # of the weights (so chunk-0 compute streams with the load)
                wq_dma(0)
                wq_dma(1)
                nxt = chunk_inputs(0)
                for g in range(2, KG):
                    wq_dma(g)

                for tq in range(NTQ):
                    tlo = tq * TQ
                    hq, cos_t, sin_t = nxt
                    if tq + 1 < NTQ:
                        nxt = chunk_inputs(tq + 1)

                    # one PSUM bank per output block (accumulation groups
                    # must be bank-exclusive)
                    pss = [qpsum.tile([128, TQ], F32, name=f"qkv_ps{nb}",
                                      tag=f"qkv_ps{nb}") for nb in range(NB)]

                    for k in range(KCH):
                        hsl = hq[k // 4][:, (k % 4) * TQ:(k % 4 + 1) * TQ]
                        for nb in range(NB):
                            nc.tensor.matmul(
                                pss[nb], wq_sl(k, nb), hsl,
                                start=(k == 0), stop=(k == KCH - 1))

                    raw5 = stpool.tile([128, NQK * TQ], F32R, name="raw5",
                                       tag="raw5")
                    # RoPE per block: drain, swap d-halves via permutation
                    # matmul on PE (DVE lanes are partition-fixed; this
                    # replaces an SBUF->SBUF DMA on the critical cadence)
                    for b in range(NQK):
                        bs = slice(b * TQ, (b + 1) * TQ)
                        nc.scalar.copy(raw5[:, bs], pss[b])
                        swp = xpsum.tile([128, TQ], F32, name="sw_ps",
                                         tag="sw_ps")
                        nc.tensor.matmul(swp, swapm, raw5[:, bs],
                                         start=True, stop=True)
                        swt = swpool.tile([128, TQ], F32, name="swt",
                                          tag="swt")
                        nc.vector.tensor_mul(swt, swp, sin_t)
                        nc.vector.tensor_mul(raw5[:, bs], raw5[:, bs], cos_t)
                        dst = q_res[b] if b < HPC else k_res
                        nc.vector.tensor_add(dst[:, tlo:tlo + TQ],
                                             raw5[:, bs], swt)
                    # V: drain, transpose 128x128 tiles on PE, store [s, d]
                    vs = vspool.tile([128, TQ], F32, name="vs", tag="vs")
                    nc.scalar.copy(vs, pss[NQK])
                    for i in range(TQ // 128):
                        tp = xpsum.tile([128, 128], F32, name="vt_ps",
                                        tag="vt_ps")
                        nc.tensor.transpose(tp, vs[:, i * 128:(i + 1) * 128],
                                            ident)
                        j = 2 * tq + i
                        nc.vector.tensor_copy(v_res[:, j * 128:(j + 1) * 128],
                                              tp)

            # ---------------- Phase 2: attention per head ------------------
            if phases >= 2:
                opool = _resctx.enter_context(
                    tc.tile_pool(name="attn_out", bufs=1))
                wopool = _resctx.enter_context(
                    tc.tile_pool(name="wo", bufs=1))
                wo = []
                for h in range(HPC):
                    wt = wopool.tile([128, HID], F32R, name=f"wo{h}",
                                     tag=f"wo{h}")
                    for wp in range(4):
                        sl = slice(wp * 1024, (wp + 1) * 1024)
                        nc.sync.dma_start(wt[:, sl],
                                          woT[h * D:(h + 1) * D, sl])
                    wo.append(wt)
                attn = []
                for h in range(HPC):
                    at = opool.tile([D, T], F32R, name=f"attn{h}",
                                    tag=f"attn{h}")
                    attn.append(at)

                with tc.tile_pool(name="pj", bufs=1) as ppool, \
                     tc.tile_pool(name="dna", bufs=2) as dpool, \
                     tc.tile_pool(name="rec", bufs=2) as rpool2, \
                     tc.tile_pool(name="scps", bufs=2, space="PSUM") as scps, \
                     tc.tile_pool(name="pvps", bufs=2, space="PSUM") as pvps:

                    # The two causal halves of a head are independent
                    # dependency chains; interleaving them (2 units of half1
                    # per unit of half0) keeps PE/Act/DVE all fed despite the
                    # serial sc -> exp -> mask -> pv chain within each unit.
                    for h in range(HPC):
                        qt = q_res[h]

                        class _Stream:
                            pass

                        def make_stream(s):
                            st = _Stream()
                            st.s = s
                            st.q_lo = 1024 * s
                            st.pv = pvps.tile([128, 1024], F32,
                                              name=f"pv{s}", tag="pv")
                            st.dna = dpool.tile([128, 1024], FP16,
                                                name=f"dna{s}", tag=f"dna{s}")
                            st.prev = None
                            st.pj_all = []
                            return st

                        def emit_pv(st, j, le, pj):
                            # far-from-diagonal pieces first: they only
                            # depend on exp, not on the tri-mask mul
                            for (plo, phi) in reversed(_pieces(le, 1024)):
                                last = (st.q_lo + phi) // 128 - 1
                                nc.tensor.matmul(
                                    st.pv[:, plo:phi],
                                    v_res[:, j * 128:(j + 1) * 128],
                                    pj[:, plo - le:phi - le],
                                    start=(j == 0), stop=(j == last))

                        def emit_unit(st, j):
                            ls = max(0, 128 * j - st.q_lo)
                            # extend so every matmul piece is >=256 wide
                            # (narrow fp32r matmuls run at 1/4 rate); the
                            # extension columns are zeroed after exp.
                            le = ls - 128 if ls % 512 == 384 else ls
                            pj = ppool.tile([128, 1024 - le], BF16,
                                            name=f"p{st.s}_{j}",
                                            tag=f"p{st.s}_{j}")
                            sc = scps.tile([128, 1024], F32,
                                           name=f"sc{st.s}", tag="sc")
                            for (plo, phi) in _pieces(le, 1024):
                                nc.tensor.matmul(
                                    sc[:, plo:phi],
                                    k_res[:, j * 128:(j + 1) * 128],
                                    qt[:, st.q_lo + plo:st.q_lo + phi],
                                    start=True, stop=True)
                            nc.scalar.activation(
                                pj, sc[:, le:1024],
                                mybir.ActivationFunctionType.Exp,
                                scale=SCALE)
                            if le < ls:
                                nc.vector.memset(pj[:, 0:ls - le], 0.0)
                            if 128 * j >= st.q_lo:
                                o = ls - le
                                nc.vector.tensor_mul(
                                    pj[:, o:o + 128], pj[:, o:o + 128], tri)
                            # software pipeline: PV of the previous j comes
                            # after this j's score matmuls, so PE always has
                            # mask-independent work queued while the
                            # Activation engine runs exp.
                            if st.prev is not None:
                                emit_pv(st, *st.prev)
                            st.prev = (j, le, pj)
                            st.pj_all.append((le, pj))

                        def finish_stream(st):
                            emit_pv(st, *st.prev)
                            # denominator column-sums batched at stream end
                            # so they stay off the per-j critical cadence
                            for i, (le, pj) in enumerate(st.pj_all):
                                if i == 0:
                                    nc.vector.tensor_copy(st.dna, pj)
                                else:
                                    nc.vector.tensor_add(
                                        st.dna[:, le:1024],
                                        st.dna[:, le:1024], pj)
                            dn = scps.tile([128, 1024], F32, name=f"dn{st.s}",
                                           tag="sc")
                            for (plo, phi) in _pieces(0, 1024):
                                nc.tensor.matmul(dn[:, plo:phi], ones,
                                                 st.dna[:, plo:phi],
                                                 start=True, stop=True)
                            rec = rpool2.tile([128, 1024], F32,
                                              name=f"rec{st.s}",
                                              tag=f"rec{st.s}")
                            nc.vector.reciprocal(rec, dn)
                            nc.vector.tensor_mul(
                                attn[h][:, st.q_lo:st.q_lo + 1024],
                                st.pv, rec)

                        for s in range(2):
                            st = make_stream(s)
                            for j in range(8 * (s + 1)):
                                emit_unit(st, j)
                            finish_stream(st)

                # ---------------- Phase 3: o_proj partial ------------------
                if phases >= 3:
                    with tc.tile_pool(name="ostage", bufs=3) as ospool, \
                         tc.tile_pool(name="ops", bufs=2,
                                      space="PSUM") as opsum:
                        for tb in range(T // 128):
                            for ch in range(2):
                                ps = opsum.tile([128, 2048], F32, name="o_ps",
                                                tag="o_ps")
                                for h in range(HPC):
                                    lhs = attn[h][:, tb * 128:(tb + 1) * 128]
                                    for cc in range(4):
                                        cl = ch * 2048 + cc * 512
                                        nc.tensor.matmul(
                                            ps[:, cc * 512:(cc + 1) * 512],
                                            lhs, wo[h][:, cl:cl + 512],
                                            start=(h == 0),
                                            stop=(h == HPC - 1))
                                ob = ospool.tile([128, 2048], BF16, name="ob",
                                                 tag="ob")
                                orow = out[tb * 128:(tb + 1) * 128,
                                           ch * 2048:(ch + 1) * 2048]
                                if tb == T // 128 - 1 and ch == 1:
                                    # pipeline the drain of the very last
                                    # tile so the kernel tail is short
                                    for q4 in range(4):
                                        sl = slice(q4 * 512, (q4 + 1) * 512)
                                        nc.scalar.copy(ob[:, sl], ps[:, sl])
                                        nc.sync.dma_start(orow[:, sl],
                                                          ob[:, sl])
                                else:
                                    nc.scalar.copy(ob, ps)
                                    for q2 in range(2):
                                        sl = slice(q2 * 1024, (q2 + 1) * 1024)
                                        nc.sync.dma_start(orow[:, sl],
                                                          ob[:, sl])
    nc.compile()
    return nc


def host_inputs(hidden_states, positions, Wqkv, Wo):
    """Build the 8 per-core input maps (host-side sharding + layout prep)."""
    f = np.float32
    hT = np.ascontiguousarray(hidden_states.T.astype(ml_dtypes.bfloat16))
    half = D // 2
    inv_freq = 1.0 / (THETA ** (np.arange(half, dtype=np.float64) / half))
    ang = inv_freq[:, None] * positions.astype(np.float64)[None, :]
    cos = np.cos(ang).astype(f)
    sin = np.sin(ang).astype(f)
    cosT = np.vstack([cos, cos])                  # [D, T]
    sinT = np.vstack([-sin, sin])                 # rotate-half sign baked in
    trim = (np.arange(128)[:, None] <= np.arange(128)[None, :]) \
        .astype(ml_dtypes.bfloat16)
    onesm = np.ones((128, 128), np.float16)

    in_maps = []
    for c in range(NCORES):
        rows = list(range(c * HPC * D, (c + 1) * HPC * D))          # Q heads
        rows += list(range(H * D + c * D, H * D + (c + 1) * D))     # K head
        rows += list(range((H + KVH) * D + c * D,
                           (H + KVH) * D + (c + 1) * D))            # V head
        wqkvT = np.ascontiguousarray(Wqkv[rows, :].T.astype(ml_dtypes.bfloat16))
        woT = np.ascontiguousarray(Wo[:, c * HPC * D:(c + 1) * HPC * D].T
                                   .astype(f))
        in_maps.append({"hT": hT, "wqkvT": wqkvT, "woT": woT,
                        "cosT": cosT, "sinT": sinT, "trim": trim,
                        "onesm": onesm})
    return in_maps


_NC_CACHE = {}


def get_nc(loop_n=1, phases=3):
    key = (loop_n, phases)
    if key not in _NC_CACHE:
        _NC_CACHE[key] = build_nc(loop_n, phases)
    return _NC_CACHE[key]


def kernel(hidden_states, positions, Wqkv, Wo, _trace=False):
    hidden_states = np.asarray(hidden_states)
    positions = np.asarray(positions)
    Wqkv = np.asarray(Wqkv)
    Wo = np.asarray(Wo)
    in_maps = host_inputs(hidden_states, positions, Wqkv, Wo)
    nc = get_nc()
    res = bass_utils.run_bass_kernel_spmd(
        nc, in_maps, core_ids=list(range(NCORES)), trace=_trace)
    acc = np.zeros((T, HID), np.float64)
    for r in res.results:
        acc += r["out"].astype(np.float64)
    out = acc.astype(np.float32)
    if _trace:
        return out, res
    return out
